# revision 4
# baseline (speedup 1.0000x reference)
"""Bass/Trainium2 kernel for BiDirectionalCrossAttention (8-core SPMD).

Sharding: 8 cores = 4 batches x 2 head-groups (4 heads each).
Per core (batch b, head-group g), v3 pipeline:
  - Q/K projections -> bias-add quantizes straight to fp8e4 in a
    DoubleRow layout ([32,2,*] per head at partition bases 0/64; DVE
    uint32-bitcast copies shuffle chans 32-63 into the sub-1 lane), so
    each score block is one fp8 DoubleRow matmul at 2x bf16 rate.
  - V projection in [token, chan] bf16 with interleaved ones-columns
    (softmax denominator falls out of the attn@V matmul for free).
  - exp on ScalarE paces the loop; attn@V accumulation stays bf16 into
    full-bank [128,512] PSUM tiles whose rows 64-127 later hold the
    PE-broadcast 1/s (ones-matmul) — no extra PSUM, no DRAM bounce.
  - Normalization chain per unit: DVE copies sums -> reciprocal (DVE
    mid-stream, ACT at the tail where the table switch is free) ->
    bf16 cast -> PE ones-broadcast into rows 64-127 -> DVE copy to
    SBUF -> DVE multiplies. All engine-local, ~5us, slack-scheduled.
  - Partial output projection Wout[:, cols_g] @ out_g -> [512, 1024].
  - Inputs stream over three DMA queues (SP/ACT/GPSIMD) so the first
    scores fire as soon as the q/k criticals land.
Host sums the two partials per batch and adds the folded bias
bout' = bout + Wout @ bv (V-bias commutes through softmax).
"""

import sys
import os

for _p in ("/opt/trn_rl_repo", "/root/.axon_site/_ro/trn_rl_repo"):
    if os.path.isdir(_p) and _p not in sys.path:
        sys.path.append(_p)

import numpy as np
import ml_dtypes

import concourse.bass as bass
import concourse.mybir as mybir
import concourse.tile as tile
from concourse.bass_utils import run_bass_kernel_spmd

BF16 = mybir.dt.bfloat16
F32 = mybir.dt.float32
FP8 = mybir.dt.float8e4
U32 = mybir.dt.uint32
NP_BF16 = ml_dtypes.bfloat16

AF = mybir.ActivationFunctionType
DR = mybir.MatmulPerfMode.DoubleRow


def _split_multi_waits(nc: bass.Bass) -> None:
    """The walrus build here allows only one sync-wait per instruction.
    Tile attaches several; hoist the extras onto same-engine NOPs placed
    immediately before the instruction (same per-engine program order)."""
    uid = 0
    for f in nc.m.functions:
        for bb in f.blocks:
            insts = bb.instructions
            out = []
            changed = False
            for inst in insts:
                si = inst.sync_info
                if si is not None and si.on_wait is not None and len(si.on_wait) > 1:
                    waits = list(si.on_wait)
                    for w in waits[:-1]:
                        nop = mybir.InstNoOp(
                            name=f"splitwait-{uid}",
                            engine=inst.engine,
                            ins=[],
                            outs=[],
                            sync_info=mybir.SyncInfo(on_wait=[w], on_update=[]),
                        )
                        uid += 1
                        out.append(nop)
                    inst.sync_info = mybir.SyncInfo(
                        on_wait=[waits[-1]], on_update=list(si.on_update or [])
                    )
                    changed = True
                out.append(inst)
            if changed:
                bb.instructions = out


def _act_reciprocal(nc: bass.Bass, out: bass.AP, in_: bass.AP) -> None:
    """ACT-engine reciprocal. bass blocks AF.Reciprocal for accuracy on
    general ranges; for our softmax sums (O(1e3), positive) it measures
    ~6e-7 rel err on HW. Used only at the tail (the Exp<->Reciprocal
    table switch costs ~1.3us on the ACT queue, hidden there)."""
    nc.scalar.add_instruction(
        mybir.InstActivation(
            name=nc.get_next_instruction_name(),
            func=AF.Reciprocal,
            ins=[
                nc.scalar.lower_ap(in_),
                mybir.ImmediateValue(dtype=F32, value=0.0),
                mybir.ImmediateValue(dtype=F32, value=1.0),
                mybir.ImmediateValue(dtype=F32, value=0.0),
            ],
            outs=[nc.scalar.lower_ap(out)],
        )
    )


def _build_program() -> bass.Bass:
    nc = bass.Bass()

    qx_d = nc.declare_dram_parameter("qx", [512, 1024], BF16, isOutput=False)
    kvx_d = nc.declare_dram_parameter("kvx", [512, 2048], BF16, isOutput=False)
    wq_d = nc.declare_dram_parameter("wq", [512, 256], BF16, isOutput=False)
    wk_d = nc.declare_dram_parameter("wk", [512, 256], BF16, isOutput=False)
    wv_d = nc.declare_dram_parameter("wv", [513, 260], BF16, isOutput=False)
    wo_d = nc.declare_dram_parameter("wo", [256, 512], BF16, isOutput=False)
    bq_d = nc.declare_dram_parameter("bq", [128, 2], F32, isOutput=False)
    bk_d = nc.declare_dram_parameter("bk", [128, 2], F32, isOutput=False)
    out_d = nc.declare_dram_parameter("out", [512, 1024], F32, isOutput=True)

    from contextlib import ExitStack

    with tile.TileContext(nc) as tc, ExitStack() as ctx:
        sb = ctx.enter_context(tc.tile_pool(name="sb", bufs=1))
        esb = ctx.enter_context(tc.tile_pool(name="esb", bufs=10))
        small = ctx.enter_context(tc.tile_pool(name="small", bufs=4))
        # PSUM budget (8 banks): "sc" 2 slots x [128,2,512] (2 banks) = 4,
        # "o" 4 slots x 1 bank = 4. Q/K-proj + out-proj borrow "o",
        # V-proj borrows "sc".
        sc_ps = ctx.enter_context(tc.tile_pool(name="scps", bufs=2, space="PSUM"))
        o_ps = ctx.enter_context(tc.tile_pool(name="ops", bufs=4, space="PSUM"))

        # ---------------- SBUF tiles ----------------
        qx_s = sb.tile([128, 4, 1024], BF16, name="qx", tag="qx")
        kvx_s = sb.tile([128, 4, 2048], BF16, name="kvx", tag="kvx")
        wq_s = sb.tile([128, 4, 256], BF16, name="wq", tag="wq")
        wk_s = sb.tile([128, 4, 256], BF16, name="wk", tag="wk")
        wv_s = sb.tile([128, 4, 260], BF16, name="wv", tag="wv")
        wv_ones = sb.tile([1, 260], BF16, name="wv_ones", tag="wv_ones")
        wo_s = sb.tile([128, 2, 512], BF16, name="wo", tag="wo")
        bq_s = sb.tile([128, 2], F32, name="bq", tag="bq")
        bk_s = sb.tile([128, 2], F32, name="bk", tag="bk")
        # ones on partitions 0 and 32 (PE-broadcast lhsT needs the base
        # partition to match the rhs row it pairs with)
        ones33 = sb.tile([33, 128], BF16, name="ones33", tag="ones33")
        qt8 = [sb.tile([128, 2, 1024], FP8, name=f"qt8{m}", tag=f"qt8{m}")
               for m in range(2)]
        kt8 = [sb.tile([128, 2, 2048], FP8, name=f"kt8{m}", tag=f"kt8{m}")
               for m in range(2)]
        v_s = sb.tile([128, 16, 260], BF16, name="v", tag="v")
        ot_s = [sb.tile([128, 1024], BF16, name=f"ot{m}", tag=f"ot{m}") for m in range(2)]

        nc.vector.memset(ones33[:], 1.0)

        # ---------------- DMAs: 3 queues, criticals first ----------------
        def chunked(d, parts=128):
            return d.rearrange("(k p) n -> p k n", p=parts)

        # SP queue: q-side criticals, then stragglers
        nc.sync.dma_start(out=wq_s[:], in_=chunked(wq_d))
        nc.sync.dma_start(out=bq_s[:], in_=bq_d[:])
        nc.sync.dma_start(out=qx_s[:, :, 0:512], in_=chunked(qx_d[:, 0:512]))
        nc.sync.dma_start(out=qx_s[:, :, 512:1024], in_=chunked(qx_d[:, 512:1024]))
        nc.sync.dma_start(out=wo_s[:], in_=wo_d.rearrange("(m p) n -> p m n", p=128))
        # ACT queue: k-side criticals only (ACT must be free for exp asap)
        nc.scalar.dma_start(out=wk_s[:], in_=chunked(wk_d))
        nc.scalar.dma_start(out=bk_s[:], in_=bk_d[:])
        nc.scalar.dma_start(out=kvx_s[:, :, 0:512], in_=chunked(kvx_d[:, 0:512]))
        # GPSIMD queue: v weights + remaining kv tokens
        nc.gpsimd.dma_start(out=wv_s[:], in_=chunked(wv_d[0:512, :]))
        nc.gpsimd.dma_start(out=wv_ones[:], in_=wv_d[512:513, :])
        for t in range(1, 4):
            nc.gpsimd.dma_start(out=kvx_s[:, :, t * 512:(t + 1) * 512],
                                in_=chunked(kvx_d[:, t * 512:(t + 1) * 512]))

        # ---------------- building blocks ----------------
        def fp8_pack(dst, psum, bias_col, tsl):
            # bias-add + fp8 quantize into sub-0 (all 128 partitions), then
            # shuffle chans 32-63 / 96-127 into sub-1 at bases 0 / 64.
            # Shuffles copy as uint32 (4 fp8/elem): DVE cost ~ free size.
            nc.vector.tensor_scalar_add(out=dst[:, 0, tsl], in0=psum,
                                        scalar1=bias_col)
            nc.vector.tensor_copy(out=dst[0:32, 1, tsl].bitcast(U32),
                                  in_=dst[32:64, 0, tsl].bitcast(U32))
            nc.vector.tensor_copy(out=dst[64:96, 1, tsl].bitcast(U32),
                                  in_=dst[96:128, 0, tsl].bitcast(U32))

        def qproj_group(m, t):
            ps = o_ps.tile([128, 512], F32, name="o", tag="o", bufs=4)
            for k in range(4):
                nc.tensor.matmul(
                    ps,
                    lhsT=wq_s[:, k, m * 128:(m + 1) * 128],
                    rhs=qx_s[:, k, t * 512:(t + 1) * 512],
                    start=(k == 0), stop=(k == 3),
                )
            fp8_pack(qt8[m], ps, bq_s[:, m:m + 1], slice(t * 512, (t + 1) * 512))

        def kproj_group(m, t):
            ps = o_ps.tile([128, 512], F32, name="o", tag="o", bufs=4)
            for k in range(4):
                nc.tensor.matmul(
                    ps,
                    lhsT=wk_s[:, k, m * 128:(m + 1) * 128],
                    rhs=kvx_s[:, k, t * 512:(t + 1) * 512],
                    start=(k == 0), stop=(k == 3),
                )
            fp8_pack(kt8[m], ps, bk_s[:, m:m + 1], slice(t * 512, (t + 1) * 512))

        def vproj_tile(tt):
            # [token,260]: cols 65j..65j+63 head-j dims, col 65j+64 == 1.0
            # (ones-row matmul; wv rows 0..511 are zero in those columns)
            ps = sc_ps.tile([128, 260], F32, name="sc", tag="sc")
            for k in range(4):
                nc.tensor.matmul(
                    ps,
                    lhsT=kvx_s[:, k, tt * 128:(tt + 1) * 128],
                    rhs=wv_s[:, k, :],
                    start=(k == 0), stop=False,
                )
            nc.tensor.matmul(
                ps,
                lhsT=ones33[0:1, :],
                rhs=wv_ones[:],
                start=False, stop=True,
            )
            nc.vector.tensor_copy(out=v_s[:, tt, :], in_=ps)

        # ---- softmax normalization: all engine-local, no DMA ----
        o_tiles = {}
        sums = {}
        recs = {}

        def norm_sums(m, t):
            oA, oB = o_tiles[(m, t)]
            # both heads' softmax sums -> [33,512] (DVE time scales with
            # free size; rows 1..31 are don't-care garbage)
            ssb = small.tile([33, 512], F32, name="ssb", tag="ssb")
            nc.vector.tensor_copy(out=ssb[0:1, :], in_=oA[64:65, :])
            nc.vector.tensor_copy(out=ssb[32:33, :], in_=oB[64:65, :])
            sums[(m, t)] = ssb

        def norm_recip(m, t, act=False):
            ssb = sums.pop((m, t))
            rec = small.tile([33, 512], F32, name="recip", tag="recip")
            if act:
                _act_reciprocal(nc, rec[:], ssb[:])
            else:
                nc.vector.reciprocal(out=rec, in_=ssb)
            rec16 = small.tile([33, 512], BF16, name="rec16", tag="rec16")
            nc.vector.tensor_copy(out=rec16[:], in_=rec[:])
            recs[(m, t)] = rec16

        def norm_bcast(m, t):
            # PE ones-matmul broadcast of 1/s into rows 64-127 of the o
            # tiles themselves (row 64's sum was consumed by norm_recip;
            # WAR dep orders the overwrite after those reads)
            oA, oB = o_tiles[(m, t)]
            rec16 = recs.pop((m, t))
            nc.tensor.matmul(
                oA[64:128, :], lhsT=ones33[0:1, 0:64], rhs=rec16[0:1, :],
                start=True, stop=True, tile_position=(0, 64),
            )
            nc.tensor.matmul(
                oB[64:128, :], lhsT=ones33[32:33, 0:64], rhs=rec16[32:33, :],
                start=True, stop=True, tile_position=(32, 64),
            )

        def norm_apply(m, t):
            qsl = slice(t * 512, (t + 1) * 512)
            oA, oB = o_tiles.pop((m, t))
            rbs = small.tile([128, 512], F32, name="rbs", tag="rbs")
            nc.vector.tensor_copy(out=rbs[0:64, :], in_=oA[64:128, :])
            nc.vector.tensor_copy(out=rbs[64:128, :], in_=oB[64:128, :])
            nc.vector.tensor_mul(ot_s[m][0:64, qsl], oA[0:64, :], rbs[0:64, :])
            nc.vector.tensor_mul(ot_s[m][64:128, qsl], oB[0:64, :], rbs[64:128, :])

        fo_tiles = {}

        def outproj_group(t2, mo, engine="vector", dma="sync"):
            if t2 not in fo_tiles:
                fo_tiles[t2] = small.tile([128, 4, 512], F32, name="fo",
                                          tag="fo", bufs=2)
            fo = fo_tiles[t2]
            ps = o_ps.tile([128, 512], F32, name="o", tag="o", bufs=4)
            for m in range(2):
                nc.tensor.matmul(
                    ps,
                    lhsT=wo_s[:, m, mo * 128:(mo + 1) * 128],
                    rhs=ot_s[m][:, t2 * 512:(t2 + 1) * 512],
                    start=(m == 0), stop=(m == 1),
                )
            if engine == "vector":
                nc.vector.tensor_copy(out=fo[:, mo, :], in_=ps)
            else:
                nc.scalar.activation(out=fo[:, mo, :], in_=ps, func=AF.Copy)
            issuer = {"sync": nc.sync, "scalar": nc.scalar, "gpsimd": nc.gpsimd}[dma]
            issuer.dma_start(
                out=out_d[mo * 128:(mo + 1) * 128, t2 * 512:(t2 + 1) * 512],
                in_=fo[:, mo, :],
            )

        # ---------------- pipelined schedule ----------------
        # 64 global iterations (4 units x 16 kv tiles); scores emitted one
        # iteration ahead so ScalarE's exp stream never waits on PE.
        units = [(0, 0), (1, 0), (0, 1), (1, 1)]
        iters = [(u, i) for u in units for i in range(16)]

        # interleave remaining projections + V tiles + norms + out-proj
        # into the per-iteration PE slack (ACT exp is the steady-state pacer)
        extra = {g: [] for g in range(64)}
        kplan = [(0, 1), (0, 2), (0, 3), (1, 0), (1, 1), (1, 2), (1, 3)]
        for idx, (m_, t_) in enumerate(kplan):
            extra[2 * idx + 1].append(lambda m_=m_, t_=t_: kproj_group(m_, t_))
        extra[0].append(lambda: qproj_group(1, 0))
        for tt in range(1, 16):
            extra[tt - 1].append(lambda tt=tt: vproj_tile(tt))
        post = {
            15: [lambda: norm_sums(0, 0)],
            16: [lambda: norm_recip(0, 0)],
            20: [lambda: norm_bcast(0, 0)],
            21: [lambda: norm_apply(0, 0)],
            23: [lambda: qproj_group(0, 1)],
            25: [lambda: qproj_group(1, 1)],
            31: [lambda: norm_sums(1, 0)],
            32: [lambda: norm_recip(1, 0)],
            36: [lambda: norm_bcast(1, 0)],
            37: [lambda: norm_apply(1, 0)],
            41: [lambda: outproj_group(0, 0)],
            43: [lambda: outproj_group(0, 1, dma="gpsimd")],
            45: [lambda: outproj_group(0, 2)],
            47: [lambda: outproj_group(0, 3, dma="gpsimd"),
                 lambda: norm_sums(0, 1)],
            48: [lambda: norm_recip(0, 1)],
            52: [lambda: norm_bcast(0, 1)],
            53: [lambda: norm_apply(0, 1)],
        }

        qproj_group(0, 0)
        kproj_group(0, 0)
        vproj_tile(0)

        sc_tiles = {}

        def emit_scores(g):
            (m, t), i = iters[g]
            ksl = slice(i * 128, (i + 1) * 128)
            qsl = slice(t * 512, (t + 1) * 512)
            sc = sc_ps.tile([128, 2, 512], F32, name="sc", tag="sc")
            nc.tensor.matmul(
                sc[:, 0, :], lhsT=kt8[m][0:32, :, ksl], rhs=qt8[m][0:32, :, qsl],
                start=True, stop=True, perf_mode=DR, tile_position=(0, 0),
            )
            nc.tensor.matmul(
                sc[:, 1, :], lhsT=kt8[m][64:96, :, ksl], rhs=qt8[m][64:96, :, qsl],
                start=True, stop=True, perf_mode=DR, tile_position=(64, 0),
            )
            sc_tiles[g] = sc

        emit_scores(0)
        for g in range(64):
            (m, t), i = iters[g]
            if g + 1 < 64:
                emit_scores(g + 1)
            sc = sc_tiles.pop(g)
            e = esb.tile([128, 2, 512], BF16, name="e", tag="e")
            nc.scalar.activation(out=e[:], in_=sc[:], func=AF.Exp, scale=0.125)
            for fn in extra.get(g, ()):
                fn()
            if i == 0:
                # full-bank tiles: rows 0-64 = attn@V out + sums, rows
                # 64-127 reused later for the PE-broadcast 1/s
                oA = o_ps.tile([128, 512], F32, name="o", tag="o", bufs=4)
                oB = o_ps.tile([128, 512], F32, name="o", tag="o", bufs=4)
                o_tiles[(m, t)] = (oA, oB)
            oA, oB = o_tiles[(m, t)]
            jA, jB = 2 * m, 2 * m + 1
            nc.tensor.matmul(
                oA[0:65, :], lhsT=v_s[:, i, 65 * jA:65 * jA + 65], rhs=e[:, 0, :],
                start=(i == 0), stop=(i == 15),
            )
            nc.tensor.matmul(
                oB[0:65, :], lhsT=v_s[:, i, 65 * jB:65 * jB + 65], rhs=e[:, 1, :],
                start=(i == 0), stop=(i == 15),
            )
            for fn in post.get(g, ()):
                fn()

        # ---------------- tail: unit (1,1) norm + out-proj ----------------
        # ACT is idle now: its reciprocal (table switch overlaps the DVE
        # sum copies) replaces the slow DVE one on the critical path.
        norm_sums(1, 1)
        norm_recip(1, 1, act=True)
        norm_bcast(1, 1)
        norm_apply(1, 1)
        outproj_group(1, 0, engine="scalar")
        outproj_group(1, 1, engine="scalar", dma="scalar")
        outproj_group(1, 2, engine="scalar", dma="gpsimd")
        outproj_group(1, 3, engine="scalar", dma="scalar")

    _split_multi_waits(nc)
    return nc


_PROGRAM = None


def _get_program() -> bass.Bass:
    global _PROGRAM
    if _PROGRAM is None:
        _PROGRAM = _build_program()
    return _PROGRAM


def _prep_core_inputs(c, q, kv, Wqkv, bqkv, Wout):
    b, g = c // 2, c % 2
    cs = slice(256 * g, 256 * g + 256)
    wv_base = Wqkv[1024 + 256 * g:1024 + 256 * g + 256, :].T  # [512, 256]
    wv = np.zeros((513, 260), np.float32)
    for j in range(4):
        wv[0:512, 65 * j:65 * j + 64] = wv_base[:, 64 * j:64 * j + 64]
        wv[512, 65 * j + 64] = 1.0
    return {
        "qx": np.ascontiguousarray(q[b].reshape(512, 1024)).astype(NP_BF16),
        "kvx": np.ascontiguousarray(kv[b].reshape(512, 2048)).astype(NP_BF16),
        "wq": np.ascontiguousarray(Wqkv[cs, :].T).astype(NP_BF16),
        "wk": np.ascontiguousarray(Wqkv[512 + 256 * g:512 + 256 * g + 256, :].T).astype(NP_BF16),
        "wv": wv.astype(NP_BF16),
        "wo": np.ascontiguousarray(Wout[:, cs].T).astype(NP_BF16),
        "bq": np.ascontiguousarray(bqkv[cs].reshape(2, 128).T).astype(np.float32),
        "bk": np.ascontiguousarray(bqkv[512 + 256 * g:512 + 256 * g + 256].reshape(2, 128).T).astype(np.float32),
    }


def kernel(q, kv, Wqkv, bqkv, Wout, bout):
    q = np.asarray(q, np.float32)
    kv = np.asarray(kv, np.float32)
    Wqkv = np.asarray(Wqkv, np.float32)
    bqkv = np.asarray(bqkv, np.float32)
    Wout = np.asarray(Wout, np.float32)
    bout = np.asarray(bout, np.float32)

    nc = _get_program()
    in_maps = [_prep_core_inputs(c, q, kv, Wqkv, bqkv, Wout) for c in range(8)]
    res = run_bass_kernel_spmd(nc, in_maps, list(range(8))).results

    # V-bias folds through softmax (rows sum to 1): bout' = bout + Wout @ bv
    bout_adj = bout + Wout @ bqkv[1024:1536]
    out = np.empty((4, 512, 32, 32), np.float32)
    for b in range(4):
        o = res[2 * b]["out"] + res[2 * b + 1]["out"] + bout_adj[:, None]
        out[b] = o.reshape(512, 32, 32)
    return out


# revision 5
# speedup vs baseline: 1.0977x; 1.0977x over previous
"""Bass/Trainium2 kernel for BiDirectionalCrossAttention (8-core SPMD).

Sharding: 8 cores = 4 batches x 2 head-groups (4 heads each).
Each core computes, for its (batch b, head-group g):
  - Q/K projections restricted to its 256 channels, channel-major [chan, token]
  - V projection in [token, chan] layout with interleaved ones-columns
    (softmax denominator falls out of the attn@V matmul for free)
  - scoresT[kv, q] per head, exp on ScalarE, attn@V accumulation on PE
    into full-bank [128,512] PSUM tiles whose rows 64-127 later hold the
    PE-broadcast 1/s (ones-matmul) — no extra PSUM, no DRAM bounce
  - normalization chain per unit: DVE copies sums -> reciprocal (DVE
    mid-stream, ACT at the tail where its table switch hides) -> bf16
    cast -> PE ones-broadcast -> DVE copy to SBUF -> DVE multiplies
  - partial output projection Wout[:, cols_g] @ out_g  -> [512, 1024]
Host sums the two partials per batch and adds the folded bias
bout' = bout + Wout @ bv (V-bias commutes through softmax since rows sum to 1).
"""

import sys
import os

for _p in ("/opt/trn_rl_repo", "/root/.axon_site/_ro/trn_rl_repo"):
    if os.path.isdir(_p) and _p not in sys.path:
        sys.path.append(_p)

import numpy as np
import ml_dtypes

import concourse.bass as bass
import concourse.mybir as mybir
import concourse.tile as tile
from concourse.bass_utils import run_bass_kernel_spmd

BF16 = mybir.dt.bfloat16
F32 = mybir.dt.float32
NP_BF16 = ml_dtypes.bfloat16

AF = mybir.ActivationFunctionType


def _split_multi_waits(nc: bass.Bass) -> None:
    """The walrus build here allows only one sync-wait per instruction.
    Tile attaches several; hoist the extras onto same-engine NOPs placed
    immediately before the instruction (same per-engine program order)."""
    uid = 0
    for f in nc.m.functions:
        for bb in f.blocks:
            insts = bb.instructions
            out = []
            changed = False
            for inst in insts:
                si = inst.sync_info
                if si is not None and si.on_wait is not None and len(si.on_wait) > 1:
                    waits = list(si.on_wait)
                    for w in waits[:-1]:
                        nop = mybir.InstNoOp(
                            name=f"splitwait-{uid}",
                            engine=inst.engine,
                            ins=[],
                            outs=[],
                            sync_info=mybir.SyncInfo(on_wait=[w], on_update=[]),
                        )
                        uid += 1
                        out.append(nop)
                    inst.sync_info = mybir.SyncInfo(
                        on_wait=[waits[-1]], on_update=list(si.on_update or [])
                    )
                    changed = True
                out.append(inst)
            if changed:
                bb.instructions = out


def _act_reciprocal(nc: bass.Bass, out: bass.AP, in_: bass.AP) -> None:
    """ACT-engine reciprocal. bass blocks AF.Reciprocal for accuracy on
    general ranges; for our softmax sums (O(1e3), positive) it measures
    ~6e-7 rel err on HW. Used only at the tail (the Exp<->Reciprocal
    table switch costs ~1.3us on the ACT queue, hidden there)."""
    nc.scalar.add_instruction(
        mybir.InstActivation(
            name=nc.get_next_instruction_name(),
            func=AF.Reciprocal,
            ins=[
                nc.scalar.lower_ap(in_),
                mybir.ImmediateValue(dtype=F32, value=0.0),
                mybir.ImmediateValue(dtype=F32, value=1.0),
                mybir.ImmediateValue(dtype=F32, value=0.0),
            ],
            outs=[nc.scalar.lower_ap(out)],
        )
    )


def _build_program() -> bass.Bass:
    nc = bass.Bass()

    qx_d = nc.declare_dram_parameter("qx", [512, 1024], BF16, isOutput=False)
    kvx_d = nc.declare_dram_parameter("kvx", [512, 2048], BF16, isOutput=False)
    wq_d = nc.declare_dram_parameter("wq", [512, 256], BF16, isOutput=False)
    wk_d = nc.declare_dram_parameter("wk", [512, 256], BF16, isOutput=False)
    wv_d = nc.declare_dram_parameter("wv", [513, 260], BF16, isOutput=False)
    wo_d = nc.declare_dram_parameter("wo", [256, 512], BF16, isOutput=False)
    bq_d = nc.declare_dram_parameter("bq", [128, 2], F32, isOutput=False)
    bk_d = nc.declare_dram_parameter("bk", [128, 2], F32, isOutput=False)
    out_d = nc.declare_dram_parameter("out", [512, 1024], F32, isOutput=True)

    from contextlib import ExitStack

    with tile.TileContext(nc) as tc, ExitStack() as ctx:
        sb = ctx.enter_context(tc.tile_pool(name="sb", bufs=1))
        esb = ctx.enter_context(tc.tile_pool(name="esb", bufs=10))
        small = ctx.enter_context(tc.tile_pool(name="small", bufs=4))
        # PSUM budget (8 banks): "sc" 2 slots x [128,2,512] (2 banks) = 4,
        # "o" 4 slots x 1 bank = 4. Q/K-proj + out-proj borrow "o", V-proj "sc".
        sc_ps = ctx.enter_context(tc.tile_pool(name="scps", bufs=2, space="PSUM"))
        o_ps = ctx.enter_context(tc.tile_pool(name="ops", bufs=4, space="PSUM"))

        # ---------------- SBUF tiles ----------------
        qx_s = sb.tile([128, 4, 1024], BF16, name="qx", tag="qx")
        kvx_s = sb.tile([128, 4, 2048], BF16, name="kvx", tag="kvx")
        wq_s = sb.tile([128, 4, 256], BF16, name="wq", tag="wq")
        wk_s = sb.tile([128, 4, 256], BF16, name="wk", tag="wk")
        wv_s = sb.tile([128, 4, 260], BF16, name="wv", tag="wv")
        wv_ones = sb.tile([1, 260], BF16, name="wv_ones", tag="wv_ones")
        wo_s = sb.tile([128, 2, 512], BF16, name="wo", tag="wo")
        bq_s = sb.tile([128, 2], F32, name="bq", tag="bq")
        bk_s = sb.tile([128, 2], F32, name="bk", tag="bk")
        # ones on partitions 0 and 32 (vproj ones-row lhsT + the PE
        # broadcast lhsT, whose base partition must match its rhs row)
        ones33 = sb.tile([33, 128], BF16, name="ones33", tag="ones33")
        qt_s = [sb.tile([128, 1024], BF16, name=f"qt{m}", tag=f"qt{m}") for m in range(2)]
        kt_s = [sb.tile([128, 2048], BF16, name=f"kt{m}", tag=f"kt{m}") for m in range(2)]
        v_s = sb.tile([128, 16, 260], BF16, name="v", tag="v")
        ot_s = [sb.tile([128, 1024], BF16, name=f"ot{m}", tag=f"ot{m}") for m in range(2)]

        nc.vector.memset(ones33[:], 1.0)

        # ---------------- DMAs, consumption order, 3D APs ----------------
        def chunked(d, parts=128):
            return d.rearrange("(k p) n -> p k n", p=parts)

        nc.sync.dma_start(out=wq_s[:], in_=chunked(wq_d))
        nc.sync.dma_start(out=bq_s[:], in_=bq_d[:])
        nc.sync.dma_start(out=qx_s[:, :, 0:512], in_=chunked(qx_d[:, 0:512]))
        nc.sync.dma_start(out=wk_s[:], in_=chunked(wk_d))
        nc.sync.dma_start(out=bk_s[:], in_=bk_d[:])
        nc.sync.dma_start(out=kvx_s[:, :, 0:512], in_=chunked(kvx_d[:, 0:512]))
        nc.sync.dma_start(out=qx_s[:, :, 512:1024], in_=chunked(qx_d[:, 512:1024]))
        nc.sync.dma_start(out=wv_s[:], in_=chunked(wv_d[0:512, :]))
        nc.sync.dma_start(out=wv_ones[:], in_=wv_d[512:513, :])
        for t in range(1, 4):
            nc.sync.dma_start(out=kvx_s[:, :, t * 512:(t + 1) * 512],
                              in_=chunked(kvx_d[:, t * 512:(t + 1) * 512]))
        nc.sync.dma_start(out=wo_s[:], in_=wo_d.rearrange("(m p) n -> p m n", p=128))

        # ---------------- building blocks ----------------
        def qproj_group(m, t):
            ps = o_ps.tile([128, 512], F32, name="o", tag="o", bufs=4)
            for k in range(4):
                nc.tensor.matmul(
                    ps,
                    lhsT=wq_s[:, k, m * 128:(m + 1) * 128],
                    rhs=qx_s[:, k, t * 512:(t + 1) * 512],
                    start=(k == 0), stop=(k == 3),
                )
            nc.vector.tensor_scalar_add(
                out=qt_s[m][:, t * 512:(t + 1) * 512], in0=ps,
                scalar1=bq_s[:, m:m + 1],
            )

        def kproj_group(m, t):
            ps = o_ps.tile([128, 512], F32, name="o", tag="o", bufs=4)
            for k in range(4):
                nc.tensor.matmul(
                    ps,
                    lhsT=wk_s[:, k, m * 128:(m + 1) * 128],
                    rhs=kvx_s[:, k, t * 512:(t + 1) * 512],
                    start=(k == 0), stop=(k == 3),
                )
            nc.vector.tensor_scalar_add(
                out=kt_s[m][:, t * 512:(t + 1) * 512], in0=ps,
                scalar1=bk_s[:, m:m + 1],
            )

        def vproj_tile(tt):
            # [token,260]: cols 65j..65j+63 head-j dims, col 65j+64 == 1.0
            # (ones-row matmul; wv rows 0..511 are zero in those columns)
            ps = sc_ps.tile([128, 260], F32, name="sc", tag="sc")
            for k in range(4):
                nc.tensor.matmul(
                    ps,
                    lhsT=kvx_s[:, k, tt * 128:(tt + 1) * 128],
                    rhs=wv_s[:, k, :],
                    start=(k == 0), stop=False,
                )
            nc.tensor.matmul(
                ps,
                lhsT=ones33[0:1, :],
                rhs=wv_ones[:],
                start=False, stop=True,
            )
            nc.vector.tensor_copy(out=v_s[:, tt, :], in_=ps)

        # ---- softmax normalization: all engine-local, no DMA ----
        o_tiles = {}
        sums = {}
        recs = {}

        def norm_sums(m, t):
            oA, oB = o_tiles[(m, t)]
            # both heads' softmax sums -> [33,512] (DVE time scales with
            # free size; rows 1..31 are don't-care garbage)
            ssb = small.tile([33, 512], F32, name="ssb", tag="ssb")
            nc.vector.tensor_copy(out=ssb[0:1, :], in_=oA[64:65, :])
            nc.vector.tensor_copy(out=ssb[32:33, :], in_=oB[64:65, :])
            sums[(m, t)] = ssb

        def norm_recip(m, t, act=False):
            ssb = sums.pop((m, t))
            rec = small.tile([33, 512], F32, name="recip", tag="recip")
            if act:
                _act_reciprocal(nc, rec[:], ssb[:])
            else:
                nc.vector.reciprocal(out=rec, in_=ssb)
            rec16 = small.tile([33, 512], BF16, name="rec16", tag="rec16")
            nc.vector.tensor_copy(out=rec16[:], in_=rec[:])
            recs[(m, t)] = rec16

        def norm_bcast(m, t):
            # PE ones-matmul broadcast of 1/s into rows 64-127 of the o
            # tiles themselves (row 64's sum was consumed by norm_recip;
            # WAR dep orders the overwrite after those reads)
            oA, oB = o_tiles[(m, t)]
            rec16 = recs.pop((m, t))
            nc.tensor.matmul(
                oA[64:128, :], lhsT=ones33[0:1, 0:64], rhs=rec16[0:1, :],
                start=True, stop=True, tile_position=(0, 64),
            )
            nc.tensor.matmul(
                oB[64:128, :], lhsT=ones33[32:33, 0:64], rhs=rec16[32:33, :],
                start=True, stop=True, tile_position=(32, 64),
            )

        def norm_apply(m, t):
            qsl = slice(t * 512, (t + 1) * 512)
            oA, oB = o_tiles.pop((m, t))
            rbs = small.tile([128, 512], F32, name="rbs", tag="rbs")
            nc.vector.tensor_copy(out=rbs[0:64, :], in_=oA[64:128, :])
            nc.vector.tensor_copy(out=rbs[64:128, :], in_=oB[64:128, :])
            nc.vector.tensor_mul(ot_s[m][0:64, qsl], oA[0:64, :], rbs[0:64, :])
            nc.vector.tensor_mul(ot_s[m][64:128, qsl], oB[0:64, :], rbs[64:128, :])

        fo_tiles = {}

        def outproj_group(t2, mo, engine="vector", dma="sync"):
            if t2 not in fo_tiles:
                fo_tiles[t2] = small.tile([128, 4, 512], F32, name="fo",
                                          tag="fo", bufs=2)
            fo = fo_tiles[t2]
            ps = o_ps.tile([128, 512], F32, name="o", tag="o", bufs=4)
            for m in range(2):
                nc.tensor.matmul(
                    ps,
                    lhsT=wo_s[:, m, mo * 128:(mo + 1) * 128],
                    rhs=ot_s[m][:, t2 * 512:(t2 + 1) * 512],
                    start=(m == 0), stop=(m == 1),
                )
            if engine == "vector":
                nc.vector.tensor_copy(out=fo[:, mo, :], in_=ps)
            else:
                nc.scalar.activation(out=fo[:, mo, :], in_=ps, func=AF.Copy)
            issuer = {"sync": nc.sync, "scalar": nc.scalar, "gpsimd": nc.gpsimd}[dma]
            issuer.dma_start(
                out=out_d[mo * 128:(mo + 1) * 128, t2 * 512:(t2 + 1) * 512],
                in_=fo[:, mo, :],
            )

        # ---------------- pipelined schedule ----------------
        # 64 global iterations (4 units x 16 kv tiles); scores emitted one
        # iteration ahead so ScalarE's exp stream never waits on PE.
        units = [(0, 0), (1, 0), (0, 1), (1, 1)]
        iters = [(u, i) for u in units for i in range(16)]

        # interleave remaining projections + V tiles + norms + out-proj
        # into the per-iteration PE slack (ACT exp is the steady-state pacer)
        extra = {g: [] for g in range(64)}
        kplan = [(0, 1), (0, 2), (0, 3), (1, 0), (1, 1), (1, 2), (1, 3)]
        for idx, (m_, t_) in enumerate(kplan):
            extra[2 * idx + 1].append(lambda m_=m_, t_=t_: kproj_group(m_, t_))
        extra[0].append(lambda: qproj_group(1, 0))
        for tt in range(16):
            extra[tt].append(lambda tt=tt: vproj_tile(tt))
        post = {
            15: [lambda: norm_sums(0, 0)],
            16: [lambda: norm_recip(0, 0)],
            20: [lambda: norm_bcast(0, 0)],
            21: [lambda: norm_apply(0, 0)],
            23: [lambda: qproj_group(0, 1)],
            25: [lambda: qproj_group(1, 1)],
            31: [lambda: norm_sums(1, 0)],
            32: [lambda: norm_recip(1, 0)],
            36: [lambda: norm_bcast(1, 0)],
            37: [lambda: norm_apply(1, 0)],
            41: [lambda: outproj_group(0, 0)],
            43: [lambda: outproj_group(0, 1, dma="gpsimd")],
            45: [lambda: outproj_group(0, 2)],
            47: [lambda: outproj_group(0, 3, dma="gpsimd"),
                 lambda: norm_sums(0, 1)],
            48: [lambda: norm_recip(0, 1)],
            52: [lambda: norm_bcast(0, 1)],
            53: [lambda: norm_apply(0, 1)],
        }

        qproj_group(0, 0)
        kproj_group(0, 0)

        sc_tiles = {}

        def emit_scores(g):
            (m, t), i = iters[g]
            ksl = slice(i * 128, (i + 1) * 128)
            qsl = slice(t * 512, (t + 1) * 512)
            sc = sc_ps.tile([128, 2, 512], F32, name="sc", tag="sc")
            nc.tensor.matmul(
                sc[:, 0, :], lhsT=kt_s[m][0:64, ksl], rhs=qt_s[m][0:64, qsl],
                start=True, stop=True, tile_position=(0, 0),
            )
            nc.tensor.matmul(
                sc[:, 1, :], lhsT=kt_s[m][64:128, ksl], rhs=qt_s[m][64:128, qsl],
                start=True, stop=True, tile_position=(64, 0),
            )
            sc_tiles[g] = sc

        emit_scores(0)
        for g in range(64):
            (m, t), i = iters[g]
            if g + 1 < 64:
                emit_scores(g + 1)
            sc = sc_tiles.pop(g)
            e = esb.tile([128, 2, 512], BF16, name="e", tag="e")
            nc.scalar.activation(out=e[:], in_=sc[:], func=AF.Exp, scale=0.125)
            for fn in extra.get(g, ()):
                fn()
            if i == 0:
                # full-bank tiles: rows 0-64 = attn@V out + sums, rows
                # 64-127 reused later for the PE-broadcast 1/s
                oA = o_ps.tile([128, 512], F32, name="o", tag="o", bufs=4)
                oB = o_ps.tile([128, 512], F32, name="o", tag="o", bufs=4)
                o_tiles[(m, t)] = (oA, oB)
            oA, oB = o_tiles[(m, t)]
            jA, jB = 2 * m, 2 * m + 1
            nc.tensor.matmul(
                oA[0:65, :], lhsT=v_s[:, i, 65 * jA:65 * jA + 65], rhs=e[:, 0, :],
                start=(i == 0), stop=(i == 15),
            )
            nc.tensor.matmul(
                oB[0:65, :], lhsT=v_s[:, i, 65 * jB:65 * jB + 65], rhs=e[:, 1, :],
                start=(i == 0), stop=(i == 15),
            )
            for fn in post.get(g, ()):
                fn()

        # ---------------- tail: unit (1,1) norm + out-proj ----------------
        # ACT is idle now: its reciprocal (table switch overlaps the DVE
        # sum copies) replaces the slow DVE one on the critical path.
        norm_sums(1, 1)
        norm_recip(1, 1, act=True)
        norm_bcast(1, 1)
        norm_apply(1, 1)
        outproj_group(1, 0, engine="scalar")
        outproj_group(1, 1, engine="scalar", dma="scalar")
        outproj_group(1, 2, engine="scalar", dma="gpsimd")
        outproj_group(1, 3, engine="scalar", dma="scalar")

    _split_multi_waits(nc)
    return nc


_PROGRAM = None


def _get_program() -> bass.Bass:
    global _PROGRAM
    if _PROGRAM is None:
        _PROGRAM = _build_program()
    return _PROGRAM


def _prep_core_inputs(c, q, kv, Wqkv, bqkv, Wout):
    b, g = c // 2, c % 2
    cs = slice(256 * g, 256 * g + 256)
    wv_base = Wqkv[1024 + 256 * g:1024 + 256 * g + 256, :].T  # [512, 256]
    wv = np.zeros((513, 260), np.float32)
    for j in range(4):
        wv[0:512, 65 * j:65 * j + 64] = wv_base[:, 64 * j:64 * j + 64]
        wv[512, 65 * j + 64] = 1.0
    return {
        "qx": np.ascontiguousarray(q[b].reshape(512, 1024)).astype(NP_BF16),
        "kvx": np.ascontiguousarray(kv[b].reshape(512, 2048)).astype(NP_BF16),
        "wq": np.ascontiguousarray(Wqkv[cs, :].T).astype(NP_BF16),
        "wk": np.ascontiguousarray(Wqkv[512 + 256 * g:512 + 256 * g + 256, :].T).astype(NP_BF16),
        "wv": wv.astype(NP_BF16),
        "wo": np.ascontiguousarray(Wout[:, cs].T).astype(NP_BF16),
        "bq": np.ascontiguousarray(bqkv[cs].reshape(2, 128).T).astype(np.float32),
        "bk": np.ascontiguousarray(bqkv[512 + 256 * g:512 + 256 * g + 256].reshape(2, 128).T).astype(np.float32),
    }


def kernel(q, kv, Wqkv, bqkv, Wout, bout):
    q = np.asarray(q, np.float32)
    kv = np.asarray(kv, np.float32)
    Wqkv = np.asarray(Wqkv, np.float32)
    bqkv = np.asarray(bqkv, np.float32)
    Wout = np.asarray(Wout, np.float32)
    bout = np.asarray(bout, np.float32)

    nc = _get_program()
    in_maps = [_prep_core_inputs(c, q, kv, Wqkv, bqkv, Wout) for c in range(8)]
    res = run_bass_kernel_spmd(nc, in_maps, list(range(8))).results

    # V-bias folds through softmax (rows sum to 1): bout' = bout + Wout @ bv
    bout_adj = bout + Wout @ bqkv[1024:1536]
    out = np.empty((4, 512, 32, 32), np.float32)
    for b in range(4):
        o = res[2 * b]["out"] + res[2 * b + 1]["out"] + bout_adj[:, None]
        out[b] = o.reshape(512, 32, 32)
    return out
